# revision 20
# baseline (speedup 1.0000x reference)
"""EdgeConv block (KNN + gather + 2-layer edge MLP + max-pool) on 8 Trainium2 cores.

Data-parallel over batch: core c processes one point cloud ([4096, 64]).

Per-core pipeline (all on device), v2:
  - negd2(i,j) = 2*x_i.x_j - |x_i|^2 - |x_j|^2 as f32r PE matmuls (1 cyc/row
    vs 4 for f32; measured |err| ~1.4e-4 rel) on 66-dim augmented vectors.
    Aug tables staged f32 then ACT-rounded to f32r (walrus requires f32r
    producers).  Diagonal killed by a DVE subtract of 1e30*I.
  - Top-16 per row: 8 chunks of 512; DVE max8 + max_index per chunk give
    top-8 candidates (end-to-end rel err of chunked candidates: 1.9e-3).
    Level 2: max8/match_replace/max8 -> tau; rp = (vals >= tau) * (N - j)
    ranked by max8 twice -> exact top-16 with lowest-j tie-break.
  - Gather via TWO InstDMAGatherAnt (1024 idx each; 2048 crashes the Q7),
    994ns+0.34ns/desc on Pool vs 16x ~1us for per-k indirect DMAs.  The
    int16 idx table needs [p%16 -> partition, replicated x8 stripes] wrap:
    built by a shuffled 4KB DRAM round-trip (SP HWDGE), a PE broadcast
    matmul (P[p,c] = M[p%16,c]), and one DVE shuffle-convert copy.
  - Edge MLP layer-1 factorized: pre1(i,k) = vg + u_i broadcast (DVE),
    GELU on ACT -> h1 bf16.  h1 PE-transposed in bf16 (1 cyc/row, bf16
    PSUM) as 8 k-pair blocks, single ACT copy each -> h1T2 [128, 1024]
    with k-parity on partition halves.  Layer-2 as 4 bf16 matmuls using
    partition bases {0,64} (W2 shipped duplicated); GELU+bias on ACT
    [128, 1024] -> h2 bf16.  Max over K: one DVE tensor_tensor across
    partition halves + one strided tensor_reduce.  PE transpose back,
    ACT->f32, HWDGE out.
"""

import sys

if "/opt/trn_rl_repo" not in sys.path:
    sys.path.insert(0, "/opt/trn_rl_repo")

import ml_dtypes
import numpy as np

import bass_rust
import concourse.bass as bass
import concourse.mybir as mybir
from concourse import library_config
from concourse.bass_utils import run_bass_kernel_spmd
from concourse.tile import TileContext
from concourse.vector_clock import ScopedClock

B, N, C, D, K = 8, 4096, 64, 64, 16
CAUG = C + 2          # augmented contraction dim for the distance matmul
NT = N // 128         # 32 i-tiles of 128 points
CH = 512              # candidate chunk length
NCH = N // CH         # 8 chunks per row
NCAND = 8 * NCH       # 64 level-1 candidates
F32 = mybir.dt.float32
F32R = mybir.dt.float32r
BF16 = mybir.dt.bfloat16
I16 = mybir.dt.int16
U16 = mybir.dt.uint16
AF = mybir.ActivationFunctionType
ALU = mybir.AluOpType

import os
DIST_F32R = int(os.environ.get("KM_DIST_F32R", "1"))   # 1: f32r (4x PE), 0: f32


class _TC(TileContext):
    """TileContext whose exit drain splits its sem waits across single-wait
    NOPs: this walrus build rejects >~2 sync waits on one SP instruction."""

    def _drain_and_barrier(self, tick_clock, wait_clock):
        gc = list(tick_clock.global_clock)
        for p, v in enumerate(gc):
            if v > 0:
                sub = [0] * len(gc)
                sub[p] = v
                nop = self.nc.sync.nop()
                wait_clock.add_sem_waits(
                    nop.ins, ScopedClock({None: bass_rust.VectorClock(sub)})
                )
        self.nc.sync.drain()
        self.nc.all_engine_barrier()
        popped = self.nc._tile_sem_poison_stack.pop()
        assert popped is self._sem_poison
        self.nc.clear_and_free_semaphores(list(self.sems.allocated().values()))
        self.nc.all_engine_barrier()


def host_constants(W1, b1, W2, b2):
    """Host-side constant tensors shipped to every core."""
    W1 = np.asarray(W1, np.float32)
    W2 = np.asarray(W2, np.float32)
    b2 = np.asarray(b2, np.float32)
    # uW applied against lhs_aug = [2x; sq; 1]: rows 0..C-1 scaled 0.5 to undo
    # the 2x, row C zero, row C+1 carries b1 (so u = x@(W1a-W1b) + b1).
    uW = np.zeros((CAUG, D), np.float32)
    uW[:C] = 0.5 * (W1[:C] - W1[C:])
    uW[C + 1] = np.asarray(b1, np.float32)
    # revb[p, f] = N - CH*(f//8): base for rev-index payloads per candidate
    revb = (N - CH * (np.arange(NCAND) // 8))[None, :] * np.ones((128, 1))
    # s16[ch, p] = 1 iff p % 16 == ch (idx-table stripe broadcast)
    s16 = (np.arange(128)[None, :] % 16 == np.arange(16)[:, None])
    consts = {
        "uW": uW,
        "vW": np.ascontiguousarray(W1[C:]),                     # [C, D]
        "W2db": np.concatenate([W2, W2], 0).astype(ml_dtypes.bfloat16),
        "b2d": np.concatenate([b2, b2]).reshape(128, 1).astype(np.float32),
        "idf": np.eye(128, dtype=np.float32),
        "idb": np.eye(128, dtype=np.float32).astype(ml_dtypes.bfloat16),
        "dgm": (1e30 * np.eye(128, dtype=np.float32)),
        "revb": revb.astype(np.float32),
        "s16": s16.astype(np.float32),
        "nonesc": -np.ones((C, 1), np.float32),
        "rone": np.ones((1, N), np.float32),
    }
    return consts


def _split_excess_waits(nc, max_waits=1):
    """Hoist excess sync waits onto same-engine NOPs (this walrus build
    rejects instructions carrying more than one sync wait)."""
    ctr = 0
    for f in nc.m.functions:
        for bb in f.blocks:
            out = []
            for ins in bb.instructions:
                si = ins.sync_info
                waits = list(si.on_wait) if si is not None and si.on_wait else []
                if len(waits) > max_waits:
                    excess, keep = waits[:-max_waits], waits[-max_waits:]
                    for i in range(0, len(excess), max_waits):
                        chunk = excess[i:i + max_waits]
                        nop = mybir.InstNoOp(
                            name=f"WS-{ctr}", engine=ins.engine, ins=[], outs=[],
                            sync_info=mybir.SyncInfo(on_wait=chunk, on_update=[]),
                        )
                        nc.register_instruction(nop, overwrite=True)
                        out.append(nop)
                        ctr += 1
                    ins.sync_info = mybir.SyncInfo(
                        on_wait=keep,
                        on_update=list(si.on_update) if si.on_update else [],
                    )
                out.append(ins)
            bb.instructions[:] = out


def build_nc(repeat=1):
    nc = bass.Bass("TRN2", target_bir_lowering=False, debug=False, num_devices=B,
                   num_swdge_queues=4, dynamic_dma_scratch_size=65536)
    x = nc.dram_tensor("x", [N, C], F32, kind="ExternalInput").ap()
    y = nc.dram_tensor("y", [N, D], F32, kind="ExternalOutput").ap()
    cin = {
        name: nc.dram_tensor(name, list(shape), dt, kind="ExternalInput").ap()
        for name, dt, shape in [
            ("uW", F32, (CAUG, D)), ("vW", F32, (C, D)),
            ("W2db", BF16, (128, D)), ("b2d", F32, (128, 1)),
            ("idf", F32, (128, 128)), ("idb", BF16, (128, 128)),
            ("dgm", F32, (128, 128)), ("revb", F32, (128, NCAND)),
            ("s16", F32, (16, 128)), ("nonesc", F32, (C, 1)),
            ("rone", F32, (1, N)),
        ]
    }

    with _TC(nc) as tc, \
         tc.tile_pool(name="const", bufs=1) as cp, \
         tc.tile_pool(name="big", bufs=1) as big, \
         tc.tile_pool(name="dram", bufs=1, space="DRAM") as dramp:
        sb = {name: cp.tile_from(ap, name=f"c_{name}") for name, ap in cin.items()}
        nc.gpsimd.load_library(library_config.mlp)
        nidx_reg = nc.gpsimd.to_reg(1024)

        DIST = F32R if DIST_F32R else F32
        rhs_r = big.tile([CAUG, N], DIST)     # [x_j; -1; -sq_j]
        lhs_r = big.tile([CAUG, N], DIST)     # [2x_i; sq_i; 1]
        u_r = big.tile([128, NT * D], F32)    # row-major u: tile t at cols [64t,)
        v_dram = dramp.tile([N, C], F32)      # row-major v table for dma_gather

        for rep in range(repeat):
            # ---------------- setup ----------------
            with tc.tile_pool(name=f"sst{rep}", bufs=1) as sst, \
                 tc.tile_pool(name=f"sup{rep}", bufs=4) as sup, \
                 tc.tile_pool(name=f"sps{rep}", bufs=2, space="PSUM") as sps, \
                 tc.tile_pool(name=f"spu{rep}", bufs=1, space="PSUM") as spu, \
                 tc.tile_pool(name=f"sxq{rep}", bufs=1) as sxq:
                if DIST_F32R:
                    rhs0 = sst.tile([CAUG, N], F32)
                    lhs0 = sst.tile([CAUG, N], F32)
                else:
                    rhs0, lhs0 = rhs_r, lhs_r
                nc.vector.memset(rhs0[C:C + 1, :], -1.0)
                nc.sync.dma_start(out=lhs0[C + 1:C + 2, :], in_=cin["rone"])
                for t in range(NT):
                    xr = sup.tile([128, C], F32, tag="xr")
                    nc.sync.dma_start(out=xr, in_=x[128 * t:128 * (t + 1), :])
                    tp = sps.tile([C, 128], F32, tag="tp")
                    nc.tensor.transpose(tp, xr, sb["idf"])
                    nc.scalar.activation(rhs0[0:C, 128 * t:128 * (t + 1)], tp, AF.Copy)
                    nc.scalar.activation(
                        lhs0[0:C, 128 * t:128 * (t + 1)], tp, AF.Copy, scale=2.0)
                xsq = sxq.tile([C, N], F32, tag="xs")
                nc.scalar.activation(xsq, rhs0[0:C, :].bitcast(F32), AF.Square)
                for h in range(2):
                    sqp = spu.tile([1, N // 2], F32, tag="uv")
                    for s in range(4):
                        c0 = 512 * s
                        nc.tensor.matmul(
                            sqp[:, c0:c0 + 512], lhsT=sb["nonesc"],
                            rhs=xsq[:, 2048 * h + c0:2048 * h + c0 + 512],
                            start=True, stop=True)
                    # sqp = -sq; +sq to lhs row 64 (legal partition base),
                    # -sq to rhs row 65 via DMA (engine APs cannot start at 65)
                    nc.scalar.activation(
                        lhs0[C:C + 1, 2048 * h:2048 * (h + 1)], sqp, AF.Copy,
                        scale=-1.0)
                    sqt = sup.tile([1, N // 2], F32, tag="sqt")
                    nc.scalar.activation(sqt, sqp, AF.Copy)
                    nc.sync.dma_start(
                        out=rhs0[C + 1:C + 2, 2048 * h:2048 * (h + 1)].bitcast(F32),
                        in_=sqt)
                # u (row-major, from lhs0 so the ones-row carries b1) and
                # v (row-major, staged through SBUF to the DRAM gather table)
                for t in range(NT):
                    i0 = 128 * t
                    upr = sps.tile([128, D], F32, tag="tp")
                    nc.tensor.matmul(upr, lhsT=lhs0[:, i0:i0 + 128].bitcast(F32),
                                     rhs=sb["uW"], start=True, stop=True)
                    nc.scalar.activation(u_r[:, D * t:D * (t + 1)], upr, AF.Copy)
                    vpr = sps.tile([128, D], F32, tag="tp")
                    nc.tensor.matmul(vpr, lhsT=rhs0[0:C, i0:i0 + 128].bitcast(F32),
                                     rhs=sb["vW"], start=True, stop=True)
                    vrow = sup.tile([128, D], F32, tag="vrow")
                    nc.scalar.activation(vrow, vpr, AF.Copy)
                    nc.sync.dma_start(out=v_dram[i0:i0 + 128, :], in_=vrow)
                if DIST_F32R:
                    # round the aug tables to f32r (walrus: f32r matmuls need
                    # f32r-rounded producers; ACT copy performs the rounding)
                    for h in range(2):
                        cs = slice(2048 * h, 2048 * (h + 1))
                        nc.scalar.activation(rhs_r[:, cs], rhs0[:, cs], AF.Copy)
                        nc.scalar.activation(lhs_r[:, cs], lhs0[:, cs], AF.Copy)

            # ---------------- main loop ----------------
            with tc.tile_pool(name=f"nd{rep}", bufs=3) as ndp, \
                 tc.tile_pool(name=f"sm{rep}", bufs=3) as smp, \
                 tc.tile_pool(name=f"ed{rep}", bufs=2) as edp, \
                 tc.tile_pool(name=f"ix{rep}", bufs=3) as ixp, \
                 tc.tile_pool(name=f"orp{rep}", bufs=3) as orp, \
                 tc.tile_pool(name=f"pq{rep}", bufs=2, space="PSUM") as pqp, \
                 tc.tile_pool(name=f"p2{rep}", bufs=1, space="PSUM") as p2p, \
                 tc.tile_pool(name=f"pib{rep}", bufs=1, space="PSUM") as pibp, \
                 tc.tile_pool(name=f"ptr{rep}", bufs=1, space="PSUM") as ptrp, \
                 tc.tile_pool(name=f"idd{rep}", bufs=3, space="DRAM") as iddp:
                pend = {}

                def stage1(t):
                    i0 = 128 * t
                    nd = ndp.tile([128, N], F32, tag="nd")
                    # distances (quarters of 1024 to double-buffer PSUM)
                    for q in range(4):
                        pq = pqp.tile([128, 1024], F32, tag="pq")
                        for s in range(2):
                            c0 = 1024 * q + 512 * s
                            nc.tensor.matmul(
                                pq[:, 512 * s:512 * (s + 1)],
                                lhsT=lhs_r[:, i0:i0 + 128],
                                rhs=rhs_r[:, c0:c0 + 512],
                                start=True, stop=True)
                        nc.scalar.activation(nd[:, 1024 * q:1024 * (q + 1)], pq,
                                             AF.Copy)
                    # self-distance kill
                    nc.vector.tensor_tensor(
                        out=nd[:, i0:i0 + 128], in0=nd[:, i0:i0 + 128],
                        in1=sb["dgm"], op=ALU.subtract)
                    # level-1 top-8 per 512-chunk
                    vals = smp.tile([128, NCAND], F32, tag="vals")
                    gidx = smp.tile([128, NCAND], U16, tag="gidx")
                    for c in range(NCH):
                        nc.vector.max(vals[:, 8 * c:8 * c + 8],
                                      nd[:, CH * c:CH * (c + 1)])
                        nc.vector.max_index(
                            gidx[:, 8 * c:8 * c + 8], vals[:, 8 * c:8 * c + 8],
                            nd[:, CH * c:CH * (c + 1)])
                    # level-2: exact top-16 with self-indexing payload
                    t8a = smp.tile([128, 8], F32, tag="t8a")
                    valsb = smp.tile([128, NCAND], F32, tag="scr")
                    t8b = smp.tile([128, 8], F32, tag="t8b")
                    nc.vector.max(t8a, vals)
                    nc.vector.match_replace(valsb, t8a, vals, -3e38)
                    nc.vector.max(t8b, valsb)
                    revi = smp.tile([128, NCAND], F32, tag="revi")
                    nc.vector.tensor_tensor(
                        out=revi, in0=sb["revb"], in1=gidx, op=ALU.subtract)
                    rp = smp.tile([128, NCAND], F32, tag="rp")
                    nc.vector.scalar_tensor_tensor(
                        out=rp, in0=vals, scalar=t8b[:, 7:8], in1=revi,
                        op0=ALU.is_ge, op1=ALU.mult)
                    rp2 = smp.tile([128, NCAND], F32, tag="scr")
                    w16 = smp.tile([128, 16], F32, tag="w16")
                    nc.vector.max(w16[:, 0:8], rp)
                    nc.vector.match_replace(rp2, w16[:, 0:8], rp, 0.0)
                    nc.vector.max(w16[:, 8:16], rp2)
                    cjf = smp.tile([128, 16], F32, tag="cjf")
                    nc.vector.tensor_scalar(
                        out=cjf, in0=w16, scalar1=-1.0, scalar2=float(N),
                        op0=ALU.mult, op1=ALU.add)
                    # idx wrap table: DRAM round-trip shuffle (i%16 -> stripe),
                    # PE stripe-broadcast, DVE col shuffle + int16 convert
                    idxd = iddp.tile([2048], F32)
                    nc.sync.dma_start(
                        out=idxd.rearrange("(ch g q) -> g ch q", ch=16, g=8, q=16),
                        in_=cjf)
                    M = ixp.tile([16, 128], F32, tag="M")
                    nc.sync.dma_start(
                        out=M, in_=idxd.rearrange("(ch c) -> ch c", ch=16))
                    Pb = pibp.tile([128, 128], F32, tag="Pb")
                    nc.tensor.matmul(Pb, lhsT=sb["s16"], rhs=M, start=True,
                                     stop=True)
                    idxs = ixp.tile([128, 128], I16, tag="idxs")
                    nc.vector.tensor_copy(
                        out=idxs.rearrange("p (h q g) -> p h q g", h=2, q=8, g=8),
                        in_=Pb.rearrange("p (g h q) -> p h q g", g=8, h=2, q=8))
                    # gather all 2048 edge v-rows as two 1024-idx batches
                    vg = edp.tile([128, K * D], F32, tag="vg")
                    vgv = vg.rearrange("p (k d) -> p k d", d=D)
                    for hh in range(2):
                        nc.gpsimd.dma_gather(
                            out_ap=vgv[:, 8 * hh:8 * (hh + 1), :],
                            in_ap=v_dram,
                            idxs_ap=idxs[:, 64 * hh:64 * (hh + 1)],
                            num_idxs=1024,
                            num_idxs_reg=nidx_reg,
                            elem_size=D,
                            queue_num=0,
                        )
                    pend[t] = (vg, vgv)

                def stage2(t):
                    i0 = 128 * t
                    vg, vgv = pend.pop(t)
                    # pre-activation: vg + u_i (broadcast over k), GELU -> bf16
                    pre1 = edp.tile([128, K * D], F32, tag="pre1")
                    ub = u_r[:, D * t:D * (t + 1)].unsqueeze(1).broadcast_to(
                        [128, K, D])
                    nc.vector.scalar_tensor_tensor(
                        out=pre1.rearrange("p (k d) -> p k d", d=D),
                        in0=vgv, scalar=1.0, in1=ub, op0=ALU.mult, op1=ALU.add)
                    h1 = edp.tile([128, K * D], BF16, tag="h1")
                    nc.scalar.activation(h1, pre1, AF.Gelu)
                    # transpose k-pair blocks (bf16 PE transpose, bf16 PSUM);
                    # two transposes share a PSUM tile -> one ACT copy each
                    h1T2 = edp.tile([128, 8 * 128], BF16, tag="h1T2")
                    for j2 in range(4):
                        tp2 = ptrp.tile([128, 256], BF16, tag="tr")
                        for jj in range(2):
                            j = 2 * j2 + jj
                            nc.tensor.transpose(
                                tp2[:, 128 * jj:128 * (jj + 1)],
                                h1[:, 128 * j:128 * (j + 1)], sb["idb"])
                        nc.scalar.activation(h1T2[:, 256 * j2:256 * (j2 + 1)], tp2,
                                             AF.Copy)
                    # layer-2: 4 bf16 matmuls, k-parity on partition halves
                    p2 = p2p.tile([128, 1024], F32, tag="p2")
                    for s in range(2):
                        cs = slice(512 * s, 512 * (s + 1))
                        nc.tensor.matmul(
                            p2[0:64, cs], lhsT=sb["W2db"][0:64, :],
                            rhs=h1T2[0:64, cs], start=True, stop=True)
                        nc.tensor.matmul(
                            p2[64:128, cs], lhsT=sb["W2db"][64:128, :],
                            rhs=h1T2[64:128, cs], start=True, stop=True)
                    # gelu into two base-0 tiles (walrus: DVE tensor_tensor
                    # requires equal SBUF base partitions)
                    h2a = edp.tile([64, 1024], BF16, tag="h2a")
                    h2b = edp.tile([64, 1024], BF16, tag="h2b")
                    nc.scalar.activation(h2a, p2[0:64, :], AF.Gelu,
                                         bias=sb["b2d"][0:64, :])
                    nc.scalar.activation(h2b, p2[64:128, :], AF.Gelu,
                                         bias=sb["b2d"][0:64, :])
                    # max over K: across k-parity halves, then a contiguous
                    # 3-level tree over j (j-major cols pair elementwise)
                    m1 = smp.tile([64, 1024], BF16, tag="m1")
                    nc.vector.tensor_tensor(out=m1, in0=h2a, in1=h2b, op=ALU.max)
                    mA = smp.tile([64, 512], BF16, tag="mA")
                    nc.vector.tensor_tensor(
                        out=mA, in0=m1[:, 0:512], in1=m1[:, 512:1024], op=ALU.max)
                    mB = smp.tile([64, 256], BF16, tag="mB")
                    nc.vector.tensor_tensor(
                        out=mB, in0=mA[:, 0:256], in1=mA[:, 256:512], op=ALU.max)
                    ot = smp.tile([64, 128], BF16, tag="ot")
                    nc.vector.tensor_tensor(
                        out=ot, in0=mB[:, 0:128], in1=mB[:, 128:256], op=ALU.max)
                    # transpose back to [128, 64] rows and store
                    otp = ptrp.tile([128, 128], BF16, tag="tr")
                    nc.tensor.transpose(otp[:, 0:64], ot, sb["idb"][0:64, 0:64])
                    orow = orp.tile([128, D], F32, tag="orow")
                    nc.scalar.activation(orow, otp[:, 0:64], AF.Copy)
                    nc.sync.dma_start(out=y[i0:i0 + 128, :], in_=orow)

                # software pipeline: the idx-shuffle/gather latency of tile t
                # hides behind tile t-1's MLP work
                for t in range(NT):
                    stage1(t)
                    if t > 0:
                        stage2(t - 1)
                stage2(NT - 1)
    mybir.codegen_inst_isa_subclasses(nc)
    _split_excess_waits(nc)
    return nc


_NC = None


def kernel(features, W1, b1, W2, b2):
    global _NC
    features = np.ascontiguousarray(np.asarray(features, np.float32))
    consts = host_constants(W1, b1, W2, b2)
    if _NC is None:
        _NC = build_nc()
    in_maps = [{"x": features[c], **consts} for c in range(B)]
    res = run_bass_kernel_spmd(_NC, in_maps, core_ids=list(range(B)))
    return np.stack([res.results[c]["y"] for c in range(B)], axis=0)


if __name__ == "__main__":
    rng = np.random.default_rng(0)
    feats = rng.standard_normal((B, N, C)).astype(np.float32)
    W1 = (rng.standard_normal((2 * C, D)) * 0.05).astype(np.float32)
    b1 = np.zeros(D, np.float32)
    W2 = (rng.standard_normal((D, D)) * 0.05).astype(np.float32)
    b2 = np.zeros(D, np.float32)
    out = kernel(features=feats, W1=W1, b1=b1, W2=W2, b2=b2)
    print(out.shape, out.dtype)


# revision 25
# speedup vs baseline: 1.0389x; 1.0389x over previous
"""EdgeConv block (KNN + gather + 2-layer edge MLP + max-pool) on 8 Trainium2 cores.

Data-parallel over batch: core c processes one point cloud ([4096, 64]).

Per-core pipeline (all on device), v2:
  - negd2(i,j) = 2*x_i.x_j - |x_i|^2 - |x_j|^2 as f32r PE matmuls (1 cyc/row
    vs 4 for f32; measured |err| ~1.4e-4 rel) on 66-dim augmented vectors.
    Aug tables staged f32 then ACT-rounded to f32r (walrus requires f32r
    producers).  Diagonal killed by a DVE subtract of 1e30*I.
  - Top-16 per row: 8 chunks of 512; DVE max8 + max_index per chunk give
    top-8 candidates (end-to-end rel err of chunked candidates: 1.9e-3).
    Level 2: max8/match_replace/max8 -> tau; rp = (vals >= tau) * (N - j)
    ranked by max8 twice -> exact top-16 with lowest-j tie-break.
  - Gather via TWO InstDMAGatherAnt (1024 idx each; 2048 crashes the Q7),
    994ns+0.34ns/desc on Pool vs 16x ~1us for per-k indirect DMAs.  The
    int16 idx table needs [p%16 -> partition, replicated x8 stripes] wrap:
    built by a shuffled 4KB DRAM round-trip (SP HWDGE), a PE broadcast
    matmul (P[p,c] = M[p%16,c]), and one DVE shuffle-convert copy.
  - Edge MLP layer-1 factorized: pre1(i,k) = vg + u_i broadcast (DVE),
    GELU on ACT -> h1 bf16.  h1 PE-transposed in bf16 (1 cyc/row, bf16
    PSUM) as 8 k-pair blocks, single ACT copy each -> h1T2 [128, 1024]
    with k-parity on partition halves.  Layer-2 as 4 bf16 matmuls using
    partition bases {0,64} (W2 shipped duplicated); GELU+bias on ACT
    [128, 1024] -> h2 bf16.  Max over K: one DVE tensor_tensor across
    partition halves + one strided tensor_reduce.  PE transpose back,
    ACT->f32, HWDGE out.
"""

import sys

if "/opt/trn_rl_repo" not in sys.path:
    sys.path.insert(0, "/opt/trn_rl_repo")

import ml_dtypes
import numpy as np

import bass_rust
import concourse.bass as bass
import concourse.mybir as mybir
from concourse import library_config
from concourse.bass_utils import run_bass_kernel_spmd
from concourse.tile import TileContext
from concourse.vector_clock import ScopedClock

B, N, C, D, K = 8, 4096, 64, 64, 16
CAUG = C + 2          # augmented contraction dim for the distance matmul
NT = N // 128         # 32 i-tiles of 128 points
CH = 512              # candidate chunk length
NCH = N // CH         # 8 chunks per row
NCAND = 8 * NCH       # 64 level-1 candidates
F32 = mybir.dt.float32
F32R = mybir.dt.float32r
BF16 = mybir.dt.bfloat16
I16 = mybir.dt.int16
U16 = mybir.dt.uint16
AF = mybir.ActivationFunctionType
ALU = mybir.AluOpType

import os
DIST_F32R = int(os.environ.get("KM_DIST_F32R", "1"))   # 1: f32r (4x PE), 0: f32


class _TC(TileContext):
    """TileContext whose exit drain splits its sem waits across single-wait
    NOPs: this walrus build rejects >~2 sync waits on one SP instruction."""

    def _drain_and_barrier(self, tick_clock, wait_clock):
        gc = list(tick_clock.global_clock)
        for p, v in enumerate(gc):
            if v > 0:
                sub = [0] * len(gc)
                sub[p] = v
                nop = self.nc.sync.nop()
                wait_clock.add_sem_waits(
                    nop.ins, ScopedClock({None: bass_rust.VectorClock(sub)})
                )
        self.nc.sync.drain()
        self.nc.all_engine_barrier()
        popped = self.nc._tile_sem_poison_stack.pop()
        assert popped is self._sem_poison
        self.nc.clear_and_free_semaphores(list(self.sems.allocated().values()))
        self.nc.all_engine_barrier()


def host_constants(W1, b1, W2, b2):
    """Host-side constant tensors shipped to every core."""
    W1 = np.asarray(W1, np.float32)
    W2 = np.asarray(W2, np.float32)
    b2 = np.asarray(b2, np.float32)
    # uW applied against lhs_aug = [2x; sq; 1]: rows 0..C-1 scaled 0.5 to undo
    # the 2x, row C zero, row C+1 carries b1 (so u = x@(W1a-W1b) + b1).
    uW = np.zeros((CAUG, D), np.float32)
    uW[:C] = 0.5 * (W1[:C] - W1[C:])
    uW[C + 1] = np.asarray(b1, np.float32)
    # revb[p, f] = N - CH*(f//8): base for rev-index payloads per candidate
    revb = (N - CH * (np.arange(NCAND) // 8))[None, :] * np.ones((128, 1))
    # s16[ch, p] = 1 iff p % 16 == ch (idx-table stripe broadcast)
    s16 = (np.arange(128)[None, :] % 16 == np.arange(16)[:, None])
    consts = {
        "uW": uW,
        "vW": np.ascontiguousarray(W1[C:]),                     # [C, D]
        "W2db": np.concatenate([W2, W2], 0).astype(ml_dtypes.bfloat16),
        "b2d": np.concatenate([b2, b2]).reshape(128, 1).astype(np.float32),
        "idf": np.eye(128, dtype=np.float32),
        "idb": np.eye(128, dtype=np.float32).astype(ml_dtypes.bfloat16),
        "dgm": (1e30 * np.eye(128, dtype=np.float32)),
        "revb": revb.astype(np.float32),
        "s16": s16.astype(np.float32),
        "nonesc": -np.ones((C, 1), np.float32),
        "rone": np.ones((1, N), np.float32),
    }
    return consts


def _split_excess_waits(nc, max_waits=1):
    """Hoist excess sync waits onto same-engine NOPs (this walrus build
    rejects instructions carrying more than one sync wait)."""
    ctr = 0
    for f in nc.m.functions:
        for bb in f.blocks:
            out = []
            for ins in bb.instructions:
                si = ins.sync_info
                waits = list(si.on_wait) if si is not None and si.on_wait else []
                if len(waits) > max_waits:
                    excess, keep = waits[:-max_waits], waits[-max_waits:]
                    for i in range(0, len(excess), max_waits):
                        chunk = excess[i:i + max_waits]
                        nop = mybir.InstNoOp(
                            name=f"WS-{ctr}", engine=ins.engine, ins=[], outs=[],
                            sync_info=mybir.SyncInfo(on_wait=chunk, on_update=[]),
                        )
                        nc.register_instruction(nop, overwrite=True)
                        out.append(nop)
                        ctr += 1
                    ins.sync_info = mybir.SyncInfo(
                        on_wait=keep,
                        on_update=list(si.on_update) if si.on_update else [],
                    )
                out.append(ins)
            bb.instructions[:] = out


def build_nc(repeat=1):
    nc = bass.Bass("TRN2", target_bir_lowering=False, debug=False, num_devices=B,
                   num_swdge_queues=4, dynamic_dma_scratch_size=65536)
    x = nc.dram_tensor("x", [N, C], F32, kind="ExternalInput").ap()
    y = nc.dram_tensor("y", [N, D], F32, kind="ExternalOutput").ap()
    cin = {
        name: nc.dram_tensor(name, list(shape), dt, kind="ExternalInput").ap()
        for name, dt, shape in [
            ("uW", F32, (CAUG, D)), ("vW", F32, (C, D)),
            ("W2db", BF16, (128, D)), ("b2d", F32, (128, 1)),
            ("idf", F32, (128, 128)), ("idb", BF16, (128, 128)),
            ("dgm", F32, (128, 128)), ("revb", F32, (128, NCAND)),
            ("s16", F32, (16, 128)), ("nonesc", F32, (C, 1)),
            ("rone", F32, (1, N)),
        ]
    }

    with _TC(nc) as tc, \
         tc.tile_pool(name="const", bufs=1) as cp, \
         tc.tile_pool(name="big", bufs=1) as big, \
         tc.tile_pool(name="dram", bufs=1, space="DRAM") as dramp:
        sb = {name: cp.tile_from(ap, name=f"c_{name}") for name, ap in cin.items()}
        nc.gpsimd.load_library(library_config.mlp)
        nidx_reg = nc.gpsimd.to_reg(1024)

        DIST = F32R if DIST_F32R else F32
        rhs_r = big.tile([CAUG, N], DIST)     # [x_j; -1; -sq_j]
        lhs_r = big.tile([CAUG, N], DIST)     # [2x_i; sq_i; 1]
        u_r = big.tile([128, NT * D], F32)    # row-major u: tile t at cols [64t,)
        v_dram = dramp.tile([N, C], F32)      # row-major v table for dma_gather

        for rep in range(repeat):
            # ---------------- setup ----------------
            with tc.tile_pool(name=f"sst{rep}", bufs=1) as sst, \
                 tc.tile_pool(name=f"sup{rep}", bufs=4) as sup, \
                 tc.tile_pool(name=f"sps{rep}", bufs=2, space="PSUM") as sps, \
                 tc.tile_pool(name=f"spu{rep}", bufs=1, space="PSUM") as spu, \
                 tc.tile_pool(name=f"sxq{rep}", bufs=1) as sxq:
                if DIST_F32R:
                    rhs0 = sst.tile([CAUG, N], F32)
                    lhs0 = sst.tile([CAUG, N], F32)
                else:
                    rhs0, lhs0 = rhs_r, lhs_r
                nc.vector.memset(rhs0[C:C + 1, :], -1.0)
                nc.sync.dma_start(out=lhs0[C + 1:C + 2, :], in_=cin["rone"])
                for t in range(NT):
                    xr = sup.tile([128, C], F32, tag="xr")
                    nc.sync.dma_start(out=xr, in_=x[128 * t:128 * (t + 1), :])
                    tp = sps.tile([C, 128], F32, tag="tp")
                    nc.tensor.transpose(tp, xr, sb["idf"])
                    nc.scalar.activation(rhs0[0:C, 128 * t:128 * (t + 1)], tp, AF.Copy)
                    nc.scalar.activation(
                        lhs0[0:C, 128 * t:128 * (t + 1)], tp, AF.Copy, scale=2.0)
                xsq = sxq.tile([C, N], F32, tag="xs")
                nc.scalar.activation(xsq, rhs0[0:C, :].bitcast(F32), AF.Square)
                for h in range(2):
                    sqp = spu.tile([1, N // 2], F32, tag="uv")
                    for s in range(4):
                        c0 = 512 * s
                        nc.tensor.matmul(
                            sqp[:, c0:c0 + 512], lhsT=sb["nonesc"],
                            rhs=xsq[:, 2048 * h + c0:2048 * h + c0 + 512],
                            start=True, stop=True)
                    # sqp = -sq; +sq to lhs row 64 (legal partition base),
                    # -sq to rhs row 65 via DMA (engine APs cannot start at 65)
                    nc.scalar.activation(
                        lhs0[C:C + 1, 2048 * h:2048 * (h + 1)], sqp, AF.Copy,
                        scale=-1.0)
                    sqt = sup.tile([1, N // 2], F32, tag="sqt")
                    nc.scalar.activation(sqt, sqp, AF.Copy)
                    nc.sync.dma_start(
                        out=rhs0[C + 1:C + 2, 2048 * h:2048 * (h + 1)].bitcast(F32),
                        in_=sqt)
                # u (row-major, from lhs0 so the ones-row carries b1) and
                # v (row-major, staged through SBUF to the DRAM gather table)
                for t in range(NT):
                    i0 = 128 * t
                    upr = sps.tile([128, D], F32, tag="tp")
                    nc.tensor.matmul(upr, lhsT=lhs0[:, i0:i0 + 128].bitcast(F32),
                                     rhs=sb["uW"], start=True, stop=True)
                    nc.scalar.activation(u_r[:, D * t:D * (t + 1)], upr, AF.Copy)
                    vpr = sps.tile([128, D], F32, tag="tp")
                    nc.tensor.matmul(vpr, lhsT=rhs0[0:C, i0:i0 + 128].bitcast(F32),
                                     rhs=sb["vW"], start=True, stop=True)
                    vrow = sup.tile([128, D], F32, tag="vrow")
                    nc.scalar.activation(vrow, vpr, AF.Copy)
                    nc.sync.dma_start(out=v_dram[i0:i0 + 128, :], in_=vrow)
                if DIST_F32R:
                    # round the aug tables to f32r (walrus: f32r matmuls need
                    # f32r-rounded producers; ACT copy performs the rounding)
                    for h in range(2):
                        cs = slice(2048 * h, 2048 * (h + 1))
                        nc.scalar.activation(rhs_r[:, cs], rhs0[:, cs], AF.Copy)
                        nc.scalar.activation(lhs_r[:, cs], lhs0[:, cs], AF.Copy)

            # ---------------- main loop ----------------
            with tc.tile_pool(name=f"nd{rep}", bufs=3) as ndp, \
                 tc.tile_pool(name=f"sm{rep}", bufs=3) as smp, \
                 tc.tile_pool(name=f"ed{rep}", bufs=2) as edp, \
                 tc.tile_pool(name=f"vgp{rep}", bufs=3) as vgp, \
                 tc.tile_pool(name=f"ix{rep}", bufs=3) as ixp, \
                 tc.tile_pool(name=f"orp{rep}", bufs=3) as orp, \
                 tc.tile_pool(name=f"pq{rep}", bufs=2, space="PSUM") as pqp, \
                 tc.tile_pool(name=f"p2{rep}", bufs=1, space="PSUM") as p2p, \
                 tc.tile_pool(name=f"pib{rep}", bufs=1, space="PSUM") as pibp, \
                 tc.tile_pool(name=f"ptr{rep}", bufs=1, space="PSUM") as ptrp, \
                 tc.tile_pool(name=f"idd{rep}", bufs=3, space="DRAM") as iddp:
                pend = {}

                def stage1(t):
                    i0 = 128 * t
                    nd = ndp.tile([128, N], F32, tag="nd")
                    # distances (quarters of 1024 to double-buffer PSUM)
                    for q in range(4):
                        pq = pqp.tile([128, 1024], F32, tag="pq")
                        for s in range(2):
                            c0 = 1024 * q + 512 * s
                            nc.tensor.matmul(
                                pq[:, 512 * s:512 * (s + 1)],
                                lhsT=lhs_r[:, i0:i0 + 128],
                                rhs=rhs_r[:, c0:c0 + 512],
                                start=True, stop=True)
                        nc.scalar.activation(nd[:, 1024 * q:1024 * (q + 1)], pq,
                                             AF.Copy)
                    # self-distance kill
                    nc.vector.tensor_tensor(
                        out=nd[:, i0:i0 + 128], in0=nd[:, i0:i0 + 128],
                        in1=sb["dgm"], op=ALU.subtract)
                    # level-1 top-8 per 512-chunk
                    vals = smp.tile([128, NCAND], F32, tag="vals")
                    gidx = smp.tile([128, NCAND], U16, tag="gidx")
                    for c in range(NCH):
                        nc.vector.max(vals[:, 8 * c:8 * c + 8],
                                      nd[:, CH * c:CH * (c + 1)])
                        nc.vector.max_index(
                            gidx[:, 8 * c:8 * c + 8], vals[:, 8 * c:8 * c + 8],
                            nd[:, CH * c:CH * (c + 1)])
                    # level-2: exact top-16 with self-indexing payload
                    t8a = smp.tile([128, 8], F32, tag="t8a")
                    valsb = smp.tile([128, NCAND], F32, tag="scr")
                    t8b = smp.tile([128, 8], F32, tag="t8b")
                    nc.vector.max(t8a, vals)
                    nc.vector.match_replace(valsb, t8a, vals, -3e38)
                    nc.vector.max(t8b, valsb)
                    revi = smp.tile([128, NCAND], F32, tag="revi")
                    nc.vector.tensor_tensor(
                        out=revi, in0=sb["revb"], in1=gidx, op=ALU.subtract)
                    rp = smp.tile([128, NCAND], F32, tag="rp")
                    nc.vector.scalar_tensor_tensor(
                        out=rp, in0=vals, scalar=t8b[:, 7:8], in1=revi,
                        op0=ALU.is_ge, op1=ALU.mult)
                    rp2 = smp.tile([128, NCAND], F32, tag="scr")
                    w16 = smp.tile([128, 16], F32, tag="w16")
                    nc.vector.max(w16[:, 0:8], rp)
                    nc.vector.match_replace(rp2, w16[:, 0:8], rp, 0.0)
                    nc.vector.max(w16[:, 8:16], rp2)
                    cjf = smp.tile([128, 16], F32, tag="cjf")
                    nc.vector.tensor_scalar(
                        out=cjf, in0=w16, scalar1=-1.0, scalar2=float(N),
                        op0=ALU.mult, op1=ALU.add)
                    # idx wrap table: DRAM round-trip shuffle (i%16 -> stripe),
                    # PE stripe-broadcast, DVE col shuffle + int16 convert
                    idxd = iddp.tile([2048], F32)
                    nc.sync.dma_start(
                        out=idxd.rearrange("(ch g q) -> g ch q", ch=16, g=8, q=16),
                        in_=cjf)
                    M = ixp.tile([16, 128], F32, tag="M")
                    nc.sync.dma_start(
                        out=M, in_=idxd.rearrange("(ch c) -> ch c", ch=16))
                    Pb = pibp.tile([128, 128], F32, tag="Pb")
                    nc.tensor.matmul(Pb, lhsT=sb["s16"], rhs=M, start=True,
                                     stop=True)
                    idxs = ixp.tile([128, 128], I16, tag="idxs")
                    nc.vector.tensor_copy(
                        out=idxs.rearrange("p (h q g) -> p h q g", h=2, q=8, g=8),
                        in_=Pb.rearrange("p (g h q) -> p h q g", g=8, h=2, q=8))
                    # gather all 2048 edge v-rows as two 1024-idx batches
                    vg = vgp.tile([128, K * D], F32, tag="vg")
                    vgv = vg.rearrange("p (k d) -> p k d", d=D)
                    for hh in range(2):
                        nc.gpsimd.dma_gather(
                            out_ap=vgv[:, 8 * hh:8 * (hh + 1), :],
                            in_ap=v_dram,
                            idxs_ap=idxs[:, 64 * hh:64 * (hh + 1)],
                            num_idxs=1024,
                            num_idxs_reg=nidx_reg,
                            elem_size=D,
                            queue_num=t % 4,
                        )
                    pend[t] = (vg, vgv)

                def stage2(t):
                    i0 = 128 * t
                    vg, vgv = pend.pop(t)
                    # pre-activation: vg + u_i (broadcast over k), GELU -> bf16
                    pre1 = edp.tile([128, K * D], F32, tag="pre1")
                    ub = u_r[:, D * t:D * (t + 1)].unsqueeze(1).broadcast_to(
                        [128, K, D])
                    nc.vector.scalar_tensor_tensor(
                        out=pre1.rearrange("p (k d) -> p k d", d=D),
                        in0=vgv, scalar=1.0, in1=ub, op0=ALU.mult, op1=ALU.add)
                    h1 = edp.tile([128, K * D], BF16, tag="h1")
                    nc.scalar.activation(h1, pre1, AF.Gelu)
                    # transpose k-pair blocks (bf16 PE transpose, bf16 PSUM);
                    # two transposes share a PSUM tile -> one ACT copy each
                    h1T2 = edp.tile([128, 8 * 128], BF16, tag="h1T2")
                    for j2 in range(4):
                        tp2 = ptrp.tile([128, 256], BF16, tag="tr")
                        for jj in range(2):
                            j = 2 * j2 + jj
                            nc.tensor.transpose(
                                tp2[:, 128 * jj:128 * (jj + 1)],
                                h1[:, 128 * j:128 * (j + 1)], sb["idb"])
                        nc.scalar.activation(h1T2[:, 256 * j2:256 * (j2 + 1)], tp2,
                                             AF.Copy)
                    # layer-2: 4 bf16 matmuls, k-parity on partition halves
                    p2 = p2p.tile([128, 1024], F32, tag="p2")
                    for s in range(2):
                        cs = slice(512 * s, 512 * (s + 1))
                        nc.tensor.matmul(
                            p2[0:64, cs], lhsT=sb["W2db"][0:64, :],
                            rhs=h1T2[0:64, cs], start=True, stop=True)
                        nc.tensor.matmul(
                            p2[64:128, cs], lhsT=sb["W2db"][64:128, :],
                            rhs=h1T2[64:128, cs], start=True, stop=True)
                    # gelu into two base-0 tiles (walrus: DVE tensor_tensor
                    # requires equal SBUF base partitions)
                    h2a = edp.tile([64, 1024], BF16, tag="h2a")
                    h2b = edp.tile([64, 1024], BF16, tag="h2b")
                    nc.scalar.activation(h2a, p2[0:64, :], AF.Gelu,
                                         bias=sb["b2d"][0:64, :])
                    nc.scalar.activation(h2b, p2[64:128, :], AF.Gelu,
                                         bias=sb["b2d"][0:64, :])
                    # max over K: across k-parity halves, then a contiguous
                    # 3-level tree over j (j-major cols pair elementwise)
                    m1 = edp.tile([64, 1024], BF16, tag="m1")
                    nc.vector.tensor_tensor(out=m1, in0=h2a, in1=h2b, op=ALU.max)
                    mA = edp.tile([64, 512], BF16, tag="mA")
                    nc.vector.tensor_tensor(
                        out=mA, in0=m1[:, 0:512], in1=m1[:, 512:1024], op=ALU.max)
                    mB = edp.tile([64, 256], BF16, tag="mB")
                    nc.vector.tensor_tensor(
                        out=mB, in0=mA[:, 0:256], in1=mA[:, 256:512], op=ALU.max)
                    ot = edp.tile([64, 128], BF16, tag="ot")
                    nc.vector.tensor_tensor(
                        out=ot, in0=mB[:, 0:128], in1=mB[:, 128:256], op=ALU.max)
                    # transpose back to [128, 64] rows and store
                    otp = ptrp.tile([128, 128], BF16, tag="tr")
                    nc.tensor.transpose(otp[:, 0:64], ot, sb["idb"][0:64, 0:64])
                    orow = orp.tile([128, D], F32, tag="orow")
                    nc.scalar.activation(orow, otp[:, 0:64], AF.Copy)
                    nc.sync.dma_start(out=y[i0:i0 + 128, :], in_=orow)

                # software pipeline, 2 deep: the idx-shuffle/gather latency of
                # tile t hides behind the MLP work of tiles t-2/t-1
                LAG = 2
                for t in range(NT):
                    stage1(t)
                    if t >= LAG:
                        stage2(t - LAG)
                for t in range(NT - LAG, NT):
                    stage2(t)
    mybir.codegen_inst_isa_subclasses(nc)
    _split_excess_waits(nc)
    return nc


_NC = None


def kernel(features, W1, b1, W2, b2):
    global _NC
    features = np.ascontiguousarray(np.asarray(features, np.float32))
    consts = host_constants(W1, b1, W2, b2)
    if _NC is None:
        _NC = build_nc()
    in_maps = [{"x": features[c], **consts} for c in range(B)]
    res = run_bass_kernel_spmd(_NC, in_maps, core_ids=list(range(B)))
    return np.stack([res.results[c]["y"] for c in range(B)], axis=0)


if __name__ == "__main__":
    rng = np.random.default_rng(0)
    feats = rng.standard_normal((B, N, C)).astype(np.float32)
    W1 = (rng.standard_normal((2 * C, D)) * 0.05).astype(np.float32)
    b1 = np.zeros(D, np.float32)
    W2 = (rng.standard_normal((D, D)) * 0.05).astype(np.float32)
    b2 = np.zeros(D, np.float32)
    out = kernel(features=feats, W1=W1, b1=b1, W2=W2, b2=b2)
    print(out.shape, out.dtype)


# revision 28
# speedup vs baseline: 2.7578x; 2.6545x over previous
"""EdgeConv block (KNN + gather + 2-layer edge MLP + max-pool) on 8 Trainium2 cores.

Data-parallel over batch: core c processes one point cloud ([4096, 64]).

Per-core pipeline (all on device), v2:
  - negd2(i,j) = 2*x_i.x_j - |x_i|^2 - |x_j|^2 as f32r PE matmuls (1 cyc/row
    vs 4 for f32; measured |err| ~1.4e-4 rel) on 66-dim augmented vectors.
    Aug tables staged f32 then ACT-rounded to f32r (walrus requires f32r
    producers).  Diagonal killed by a DVE subtract of 1e30*I.
  - Top-16 per row: 8 chunks of 512; DVE max8 + max_index per chunk give
    top-8 candidates (end-to-end rel err of chunked candidates: 1.9e-3).
    Level 2: max8/match_replace/max8 -> tau; rp = (vals >= tau) * (N - j)
    ranked by max8 twice -> exact top-16 with lowest-j tie-break.
  - Gather via TWO InstDMAGatherAnt (1024 idx each; 2048 crashes the Q7),
    994ns+0.34ns/desc on Pool vs 16x ~1us for per-k indirect DMAs.  The
    int16 idx table needs [p%16 -> partition, replicated x8 stripes] wrap:
    built by a shuffled 4KB DRAM round-trip (SP HWDGE), a PE broadcast
    matmul (P[p,c] = M[p%16,c]), and one DVE shuffle-convert copy.
  - Edge MLP layer-1 factorized: pre1(i,k) = vg + u_i broadcast (DVE),
    GELU on ACT -> h1 bf16.  h1 PE-transposed in bf16 (1 cyc/row, bf16
    PSUM) as 8 k-pair blocks, single ACT copy each -> h1T2 [128, 1024]
    with k-parity on partition halves.  Layer-2 as 4 bf16 matmuls using
    partition bases {0,64} (W2 shipped duplicated); GELU+bias on ACT
    [128, 1024] -> h2 bf16 (two base-0 tiles; walrus requires equal SBUF
    base partitions for DVE tensor_tensor).  Max over K: one DVE
    tensor_tensor across k-parity tiles + a contiguous 3-level bf16 TT
    tree over j.  PE transpose back, ACT->f32, HWDGE out.

  The i-tile loop is software-pipelined 2 deep (stage1: distance/topk/idx/
  gather; stage2: MLP) so the ~12us idx-shuffle+gather latency of tile t
  hides behind the MLP work of tiles t-2/t-1.
"""

import sys

if "/opt/trn_rl_repo" not in sys.path:
    sys.path.insert(0, "/opt/trn_rl_repo")

import ml_dtypes
import numpy as np

import bass_rust
import concourse.bass as bass
import concourse.mybir as mybir
from concourse import library_config
from concourse.bass_utils import run_bass_kernel_spmd
from concourse.tile import TileContext
from concourse.vector_clock import ScopedClock

B, N, C, D, K = 8, 4096, 64, 64, 16
CAUG = C + 2          # augmented contraction dim for the distance matmul
NT = N // 128         # 32 i-tiles of 128 points
CH = 512              # candidate chunk length
NCH = N // CH         # 8 chunks per row
NCAND = 8 * NCH       # 64 level-1 candidates
F32 = mybir.dt.float32
F32R = mybir.dt.float32r
BF16 = mybir.dt.bfloat16
I16 = mybir.dt.int16
U16 = mybir.dt.uint16
AF = mybir.ActivationFunctionType
ALU = mybir.AluOpType

import os
DIST_F32R = int(os.environ.get("KM_DIST_F32R", "1"))   # 1: f32r (4x PE), 0: f32


class _TC(TileContext):
    """TileContext whose exit drain splits its sem waits across single-wait
    NOPs: this walrus build rejects >~2 sync waits on one SP instruction."""

    def _drain_and_barrier(self, tick_clock, wait_clock):
        gc = list(tick_clock.global_clock)
        for p, v in enumerate(gc):
            if v > 0:
                sub = [0] * len(gc)
                sub[p] = v
                nop = self.nc.sync.nop()
                wait_clock.add_sem_waits(
                    nop.ins, ScopedClock({None: bass_rust.VectorClock(sub)})
                )
        self.nc.sync.drain()
        self.nc.all_engine_barrier()
        popped = self.nc._tile_sem_poison_stack.pop()
        assert popped is self._sem_poison
        self.nc.clear_and_free_semaphores(list(self.sems.allocated().values()))
        self.nc.all_engine_barrier()


def host_constants(W1, b1, W2, b2):
    """Host-side constant tensors shipped to every core."""
    W1 = np.asarray(W1, np.float32)
    W2 = np.asarray(W2, np.float32)
    b2 = np.asarray(b2, np.float32)
    # uW applied against lhs_aug = [2x; sq; 1]: rows 0..C-1 scaled 0.5 to undo
    # the 2x, row C zero, row C+1 carries b1 (so u = x@(W1a-W1b) + b1).
    uW = np.zeros((CAUG, D), np.float32)
    uW[:C] = 0.5 * (W1[:C] - W1[C:])
    uW[C + 1] = np.asarray(b1, np.float32)
    # revb[p, f] = N - CH*(f//8): base for rev-index payloads per candidate
    revb = (N - CH * (np.arange(NCAND) // 8))[None, :] * np.ones((128, 1))
    # s16[ch, p] = 1 iff p % 16 == ch (idx-table stripe broadcast)
    s16 = (np.arange(128)[None, :] % 16 == np.arange(16)[:, None])
    consts = {
        "uW": uW,
        "vW": np.ascontiguousarray(W1[C:]),                     # [C, D]
        "W2db": np.concatenate([W2, W2], 0).astype(ml_dtypes.bfloat16),
        "b2d": np.concatenate([b2, b2]).reshape(128, 1).astype(np.float32),
        "idf": np.eye(128, dtype=np.float32),
        "idb": np.eye(128, dtype=np.float32).astype(ml_dtypes.bfloat16),
        "dgm": (1e30 * np.eye(128, dtype=np.float32)),
        "revb": revb.astype(np.float32),
        "s16": s16.astype(np.float32),
        "nonesc": -np.ones((C, 1), np.float32),
        "rone": np.ones((1, N), np.float32),
    }
    return consts


def _split_excess_waits(nc, max_waits=1):
    """Hoist excess sync waits onto same-engine NOPs (this walrus build
    rejects instructions carrying more than one sync wait)."""
    ctr = 0
    for f in nc.m.functions:
        for bb in f.blocks:
            out = []
            for ins in bb.instructions:
                si = ins.sync_info
                waits = list(si.on_wait) if si is not None and si.on_wait else []
                if len(waits) > max_waits:
                    excess, keep = waits[:-max_waits], waits[-max_waits:]
                    for i in range(0, len(excess), max_waits):
                        chunk = excess[i:i + max_waits]
                        nop = mybir.InstNoOp(
                            name=f"WS-{ctr}", engine=ins.engine, ins=[], outs=[],
                            sync_info=mybir.SyncInfo(on_wait=chunk, on_update=[]),
                        )
                        nc.register_instruction(nop, overwrite=True)
                        out.append(nop)
                        ctr += 1
                    ins.sync_info = mybir.SyncInfo(
                        on_wait=keep,
                        on_update=list(si.on_update) if si.on_update else [],
                    )
                out.append(ins)
            bb.instructions[:] = out


def build_nc(repeat=1):
    nc = bass.Bass("TRN2", target_bir_lowering=False, debug=False, num_devices=B,
                   num_swdge_queues=4, dynamic_dma_scratch_size=65536)
    x = nc.dram_tensor("x", [N, C], F32, kind="ExternalInput").ap()
    y = nc.dram_tensor("y", [N, D], F32, kind="ExternalOutput").ap()
    cin = {
        name: nc.dram_tensor(name, list(shape), dt, kind="ExternalInput").ap()
        for name, dt, shape in [
            ("uW", F32, (CAUG, D)), ("vW", F32, (C, D)),
            ("W2db", BF16, (128, D)), ("b2d", F32, (128, 1)),
            ("idf", F32, (128, 128)), ("idb", BF16, (128, 128)),
            ("dgm", F32, (128, 128)), ("revb", F32, (128, NCAND)),
            ("s16", F32, (16, 128)), ("nonesc", F32, (C, 1)),
            ("rone", F32, (1, N)),
        ]
    }

    with _TC(nc) as tc, \
         tc.tile_pool(name="const", bufs=1) as cp, \
         tc.tile_pool(name="big", bufs=1) as big, \
         tc.tile_pool(name="dram", bufs=1, space="DRAM") as dramp:
        sb = {name: cp.tile_from(ap, name=f"c_{name}") for name, ap in cin.items()}
        nc.gpsimd.load_library(library_config.mlp)
        nidx_reg = nc.gpsimd.to_reg(1024)

        DIST = F32R if DIST_F32R else F32
        rhs_r = big.tile([CAUG, N], DIST)     # [x_j; -1; -sq_j]
        lhs_r = big.tile([CAUG, N], DIST)     # [2x_i; sq_i; 1]
        u_r = big.tile([128, NT * D], F32)    # row-major u: tile t at cols [64t,)
        v_dram = dramp.tile([N, C], F32)      # row-major v table for dma_gather

        for rep in range(repeat):
            # ---------------- setup ----------------
            with tc.tile_pool(name=f"sst{rep}", bufs=1) as sst, \
                 tc.tile_pool(name=f"sup{rep}", bufs=4) as sup, \
                 tc.tile_pool(name=f"sps{rep}", bufs=2, space="PSUM") as sps, \
                 tc.tile_pool(name=f"spu{rep}", bufs=1, space="PSUM") as spu, \
                 tc.tile_pool(name=f"sxq{rep}", bufs=1) as sxq:
                if DIST_F32R:
                    rhs0 = sst.tile([CAUG, N], F32)
                    lhs0 = sst.tile([CAUG, N], F32)
                else:
                    rhs0, lhs0 = rhs_r, lhs_r
                nc.vector.memset(rhs0[C:C + 1, :], -1.0)
                nc.sync.dma_start(out=lhs0[C + 1:C + 2, :], in_=cin["rone"])
                for t in range(NT):
                    xr = sup.tile([128, C], F32, tag="xr")
                    nc.sync.dma_start(out=xr, in_=x[128 * t:128 * (t + 1), :])
                    tp = sps.tile([C, 128], F32, tag="tp")
                    nc.tensor.transpose(tp, xr, sb["idf"])
                    cs = slice(128 * t, 128 * (t + 1))
                    # alternate engines so setup copies run on ACT and DVE in
                    # parallel (DVE is otherwise idle until the first L1)
                    if t % 2 == 0:
                        nc.scalar.activation(rhs0[0:C, cs], tp, AF.Copy)
                        nc.scalar.activation(lhs0[0:C, cs], tp, AF.Copy, scale=2.0)
                    else:
                        nc.vector.tensor_copy(rhs0[0:C, cs].bitcast(F32), tp)
                        nc.vector.tensor_scalar(
                            out=lhs0[0:C, cs].bitcast(F32), in0=tp, scalar1=2.0,
                            scalar2=None, op0=ALU.mult)
                xsq = sxq.tile([C, N], F32, tag="xs")
                nc.scalar.activation(xsq, rhs0[0:C, :].bitcast(F32), AF.Square)
                for h in range(2):
                    sqp = spu.tile([1, N // 2], F32, tag="uv")
                    for s in range(4):
                        c0 = 512 * s
                        nc.tensor.matmul(
                            sqp[:, c0:c0 + 512], lhsT=sb["nonesc"],
                            rhs=xsq[:, 2048 * h + c0:2048 * h + c0 + 512],
                            start=True, stop=True)
                    # sqp = -sq; +sq to lhs row 64 (legal partition base),
                    # -sq to rhs row 65 via DMA (engine APs cannot start at 65)
                    nc.scalar.activation(
                        lhs0[C:C + 1, 2048 * h:2048 * (h + 1)], sqp, AF.Copy,
                        scale=-1.0)
                    sqt = sup.tile([1, N // 2], F32, tag="sqt")
                    nc.scalar.activation(sqt, sqp, AF.Copy)
                    nc.sync.dma_start(
                        out=rhs0[C + 1:C + 2, 2048 * h:2048 * (h + 1)].bitcast(F32),
                        in_=sqt)
                # u (row-major, from lhs0 so the ones-row carries b1) and
                # v (row-major, staged through SBUF to the DRAM gather table)
                for t in range(NT):
                    i0 = 128 * t
                    upr = sps.tile([128, D], F32, tag="tp")
                    nc.tensor.matmul(upr, lhsT=lhs0[:, i0:i0 + 128].bitcast(F32),
                                     rhs=sb["uW"], start=True, stop=True)
                    if t % 2 == 0:
                        nc.scalar.activation(u_r[:, D * t:D * (t + 1)], upr,
                                             AF.Copy)
                    else:
                        nc.vector.tensor_copy(u_r[:, D * t:D * (t + 1)], upr)
                    vpr = sps.tile([128, D], F32, tag="tp")
                    nc.tensor.matmul(vpr, lhsT=rhs0[0:C, i0:i0 + 128].bitcast(F32),
                                     rhs=sb["vW"], start=True, stop=True)
                    vrow = sup.tile([128, D], F32, tag="vrow")
                    if t % 2 == 0:
                        nc.scalar.activation(vrow, vpr, AF.Copy)
                    else:
                        nc.vector.tensor_copy(vrow, vpr)
                    nc.sync.dma_start(out=v_dram[i0:i0 + 128, :], in_=vrow)
                if DIST_F32R:
                    # round the aug tables to f32r (walrus: f32r matmuls need
                    # f32r-rounded producers; ACT copy performs the rounding)
                    for h in range(2):
                        cs = slice(2048 * h, 2048 * (h + 1))
                        nc.scalar.activation(rhs_r[:, cs], rhs0[:, cs], AF.Copy)
                        nc.scalar.activation(lhs_r[:, cs], lhs0[:, cs], AF.Copy)

            # ---------------- main loop ----------------
            with tc.tile_pool(name=f"nd{rep}", bufs=3) as ndp, \
                 tc.tile_pool(name=f"sm{rep}", bufs=3) as smp, \
                 tc.tile_pool(name=f"ed{rep}", bufs=2) as edp, \
                 tc.tile_pool(name=f"vgp{rep}", bufs=3) as vgp, \
                 tc.tile_pool(name=f"ix{rep}", bufs=3) as ixp, \
                 tc.tile_pool(name=f"orp{rep}", bufs=3) as orp, \
                 tc.tile_pool(name=f"pq{rep}", bufs=2, space="PSUM") as pqp, \
                 tc.tile_pool(name=f"p2{rep}", bufs=1, space="PSUM") as p2p, \
                 tc.tile_pool(name=f"pib{rep}", bufs=1, space="PSUM") as pibp, \
                 tc.tile_pool(name=f"ptr{rep}", bufs=1, space="PSUM") as ptrp, \
                 tc.tile_pool(name=f"idd{rep}", bufs=3, space="DRAM") as iddp:
                pend = {}

                def stage1(t):
                    i0 = 128 * t
                    nd = ndp.tile([128, N], F32, tag="nd")
                    # distances (quarters of 1024 to double-buffer PSUM)
                    for q in range(4):
                        pq = pqp.tile([128, 1024], F32, tag="pq")
                        for s in range(2):
                            c0 = 1024 * q + 512 * s
                            nc.tensor.matmul(
                                pq[:, 512 * s:512 * (s + 1)],
                                lhsT=lhs_r[:, i0:i0 + 128],
                                rhs=rhs_r[:, c0:c0 + 512],
                                start=True, stop=True)
                        nc.scalar.activation(nd[:, 1024 * q:1024 * (q + 1)], pq,
                                             AF.Copy)
                    # self-distance kill
                    nc.vector.tensor_tensor(
                        out=nd[:, i0:i0 + 128], in0=nd[:, i0:i0 + 128],
                        in1=sb["dgm"], op=ALU.subtract)
                    # level-1 top-8 per 512-chunk
                    vals = smp.tile([128, NCAND], F32, tag="vals")
                    gidx = smp.tile([128, NCAND], U16, tag="gidx")
                    for c in range(NCH):
                        nc.vector.max(vals[:, 8 * c:8 * c + 8],
                                      nd[:, CH * c:CH * (c + 1)])
                        nc.vector.max_index(
                            gidx[:, 8 * c:8 * c + 8], vals[:, 8 * c:8 * c + 8],
                            nd[:, CH * c:CH * (c + 1)])
                    # level-2: exact top-16 with self-indexing payload
                    t8a = smp.tile([128, 8], F32, tag="t8a")
                    valsb = smp.tile([128, NCAND], F32, tag="scr")
                    t8b = smp.tile([128, 8], F32, tag="t8b")
                    nc.vector.max(t8a, vals)
                    nc.vector.match_replace(valsb, t8a, vals, -3e38)
                    nc.vector.max(t8b, valsb)
                    revi = smp.tile([128, NCAND], F32, tag="revi")
                    nc.vector.tensor_tensor(
                        out=revi, in0=sb["revb"], in1=gidx, op=ALU.subtract)
                    rp = smp.tile([128, NCAND], F32, tag="rp")
                    nc.vector.scalar_tensor_tensor(
                        out=rp, in0=vals, scalar=t8b[:, 7:8], in1=revi,
                        op0=ALU.is_ge, op1=ALU.mult)
                    rp2 = smp.tile([128, NCAND], F32, tag="scr")
                    w16 = smp.tile([128, 16], F32, tag="w16")
                    nc.vector.max(w16[:, 0:8], rp)
                    nc.vector.match_replace(rp2, w16[:, 0:8], rp, 0.0)
                    nc.vector.max(w16[:, 8:16], rp2)
                    cjf = smp.tile([128, 16], F32, tag="cjf")
                    nc.vector.tensor_scalar(
                        out=cjf, in0=w16, scalar1=-1.0, scalar2=float(N),
                        op0=ALU.mult, op1=ALU.add)
                    # idx wrap table: DRAM round-trip shuffle (i%16 -> stripe),
                    # PE stripe-broadcast, DVE col shuffle + int16 convert
                    idxd = iddp.tile([2048], F32)
                    nc.sync.dma_start(
                        out=idxd.rearrange("(ch g q) -> g ch q", ch=16, g=8, q=16),
                        in_=cjf)
                    M = ixp.tile([16, 128], F32, tag="M")
                    nc.sync.dma_start(
                        out=M, in_=idxd.rearrange("(ch c) -> ch c", ch=16))
                    Pb = pibp.tile([128, 128], F32, tag="Pb")
                    nc.tensor.matmul(Pb, lhsT=sb["s16"], rhs=M, start=True,
                                     stop=True)
                    idxs = ixp.tile([128, 128], I16, tag="idxs")
                    nc.vector.tensor_copy(
                        out=idxs.rearrange("p (h q g) -> p h q g", h=2, q=8, g=8),
                        in_=Pb.rearrange("p (g h q) -> p h q g", g=8, h=2, q=8))
                    # gather all 2048 edge v-rows as two 1024-idx batches
                    vg = vgp.tile([128, K * D], F32, tag="vg")
                    vgv = vg.rearrange("p (k d) -> p k d", d=D)
                    for hh in range(2):
                        nc.gpsimd.dma_gather(
                            out_ap=vgv[:, 8 * hh:8 * (hh + 1), :],
                            in_ap=v_dram,
                            idxs_ap=idxs[:, 64 * hh:64 * (hh + 1)],
                            num_idxs=1024,
                            num_idxs_reg=nidx_reg,
                            elem_size=D,
                            queue_num=t % 4,
                        )
                    pend[t] = (vg, vgv)

                def stage2(t):
                    i0 = 128 * t
                    vg, vgv = pend.pop(t)
                    # pre-activation: vg + u_i (broadcast over k), GELU -> bf16
                    pre1 = edp.tile([128, K * D], F32, tag="pre1")
                    ub = u_r[:, D * t:D * (t + 1)].unsqueeze(1).broadcast_to(
                        [128, K, D])
                    nc.vector.scalar_tensor_tensor(
                        out=pre1.rearrange("p (k d) -> p k d", d=D),
                        in0=vgv, scalar=1.0, in1=ub, op0=ALU.mult, op1=ALU.add)
                    h1 = edp.tile([128, K * D], BF16, tag="h1")
                    nc.scalar.activation(h1, pre1, AF.Gelu)
                    # transpose k-pair blocks (bf16 PE transpose, bf16 PSUM);
                    # two transposes share a PSUM tile -> one ACT copy each
                    h1T2 = edp.tile([128, 8 * 128], BF16, tag="h1T2")
                    for j2 in range(4):
                        tp2 = ptrp.tile([128, 256], BF16, tag="tr")
                        for jj in range(2):
                            j = 2 * j2 + jj
                            nc.tensor.transpose(
                                tp2[:, 128 * jj:128 * (jj + 1)],
                                h1[:, 128 * j:128 * (j + 1)], sb["idb"])
                        nc.scalar.activation(h1T2[:, 256 * j2:256 * (j2 + 1)], tp2,
                                             AF.Copy)
                    # layer-2: 4 bf16 matmuls, k-parity on partition halves
                    p2 = p2p.tile([128, 1024], F32, tag="p2")
                    for s in range(2):
                        cs = slice(512 * s, 512 * (s + 1))
                        nc.tensor.matmul(
                            p2[0:64, cs], lhsT=sb["W2db"][0:64, :],
                            rhs=h1T2[0:64, cs], start=True, stop=True)
                        nc.tensor.matmul(
                            p2[64:128, cs], lhsT=sb["W2db"][64:128, :],
                            rhs=h1T2[64:128, cs], start=True, stop=True)
                    # gelu into two base-0 tiles (walrus: DVE tensor_tensor
                    # requires equal SBUF base partitions)
                    h2a = edp.tile([64, 1024], BF16, tag="h2a")
                    h2b = edp.tile([64, 1024], BF16, tag="h2b")
                    nc.scalar.activation(h2a, p2[0:64, :], AF.Gelu,
                                         bias=sb["b2d"][0:64, :])
                    nc.scalar.activation(h2b, p2[64:128, :], AF.Gelu,
                                         bias=sb["b2d"][0:64, :])
                    # max over K: across k-parity halves, then a contiguous
                    # 3-level tree over j (j-major cols pair elementwise)
                    m1 = edp.tile([64, 1024], BF16, tag="m1")
                    nc.vector.tensor_tensor(out=m1, in0=h2a, in1=h2b, op=ALU.max)
                    mA = edp.tile([64, 512], BF16, tag="mA")
                    nc.vector.tensor_tensor(
                        out=mA, in0=m1[:, 0:512], in1=m1[:, 512:1024], op=ALU.max)
                    mB = edp.tile([64, 256], BF16, tag="mB")
                    nc.vector.tensor_tensor(
                        out=mB, in0=mA[:, 0:256], in1=mA[:, 256:512], op=ALU.max)
                    ot = edp.tile([64, 128], BF16, tag="ot")
                    nc.vector.tensor_tensor(
                        out=ot, in0=mB[:, 0:128], in1=mB[:, 128:256], op=ALU.max)
                    # transpose back to [128, 64] rows and store
                    otp = ptrp.tile([128, 128], BF16, tag="tr")
                    nc.tensor.transpose(otp[:, 0:64], ot, sb["idb"][0:64, 0:64])
                    orow = orp.tile([128, D], F32, tag="orow")
                    nc.scalar.activation(orow, otp[:, 0:64], AF.Copy)
                    nc.sync.dma_start(out=y[i0:i0 + 128, :], in_=orow)

                # software pipeline, 2 deep: the idx-shuffle/gather latency of
                # tile t hides behind the MLP work of tiles t-2/t-1
                LAG = 2
                for t in range(NT):
                    stage1(t)
                    if t >= LAG:
                        stage2(t - LAG)
                for t in range(NT - LAG, NT):
                    stage2(t)
    mybir.codegen_inst_isa_subclasses(nc)
    _split_excess_waits(nc)
    return nc


_NC = None


def kernel(features, W1, b1, W2, b2):
    global _NC
    features = np.ascontiguousarray(np.asarray(features, np.float32))
    consts = host_constants(W1, b1, W2, b2)
    if _NC is None:
        _NC = build_nc()
    in_maps = [{"x": features[c], **consts} for c in range(B)]
    res = run_bass_kernel_spmd(_NC, in_maps, core_ids=list(range(B)))
    return np.stack([res.results[c]["y"] for c in range(B)], axis=0)


if __name__ == "__main__":
    rng = np.random.default_rng(0)
    feats = rng.standard_normal((B, N, C)).astype(np.float32)
    W1 = (rng.standard_normal((2 * C, D)) * 0.05).astype(np.float32)
    b1 = np.zeros(D, np.float32)
    W2 = (rng.standard_normal((D, D)) * 0.05).astype(np.float32)
    b2 = np.zeros(D, np.float32)
    out = kernel(features=feats, W1=W1, b1=b1, W2=W2, b2=b2)
    print(out.shape, out.dtype)
